# revision 20
# baseline (speedup 1.0000x reference)
"""GCN forward on 8 TRN2 NeuronCores — slot-aligned gather + strided-reduce design.

Model (reference.py): 2-layer GCN, N=100000 nodes, E=1600000 edges,
IN=HID=128, OUT=64, symmetric normalization with self-loops.

Math (dinv = (in_deg+1)^-1/2, folded on host where linear):
  table1[s] = (dinv*x)[s] @ W1            (x pre-scaled on host)
  y1[t]     = relu(dinv[t] * sum_e table1[src_e] + b1)
  table2[s] = dinv[s] * y1[s]
  out[t]    = dinv[t] * (sum_e table2[src_e]) @ W2 + b2

Device layout: targets of each core are permuted (degree-balanced) into
(group g in [0,98), slot s in [0,128)) positions. The per-edge gather
stream is slot-aligned — the edge for slot s sits at stream position
≡ s (mod 128) — so aggregation is a single strided tensor_reduce per
slab of groups instead of per-block one-hot matmuls.
"""

import sys

sys.path.insert(0, "/opt/trn_rl_repo")
import numpy as np
import ml_dtypes

import concourse.bass as bass
import concourse.mybir as mybir
import concourse.tile as tile
from concourse import bacc
from concourse.bass_utils import run_bass_kernel_spmd

F32 = mybir.dt.float32
BF16 = mybir.dt.bfloat16
I16 = mybir.dt.int16
AF = mybir.ActivationFunctionType
ALU = mybir.AluOpType

P = 128
N, E = 100000, 1600000
IN, HID, OUT = 128, 128, 64
NCORES = 8
TPC = 12500
PPC = 12544
NPAD = PPC * NCORES
NCHUNK = 4
CHUNK = NPAD // NCHUNK  # 25088 rows per gather window (int16 index range)
NG = PPC // P  # 98 target groups per core
SLABB = 56  # max ng*nbu per reduce slab (bounds the msgs tile at 56KB/part)
PADIDX = 12500  # a guaranteed-zero table row inside every chunk


def host_prep(edge_index):
    row = np.asarray(edge_index[0], dtype=np.int64)
    col = np.asarray(edge_index[1], dtype=np.int64)
    loop = np.arange(N, dtype=np.int64)
    src = np.concatenate([row, loop])
    tgt = np.concatenate([col, loop])
    deg = np.bincount(tgt, minlength=N)
    dinv = 1.0 / np.sqrt(deg.astype(np.float64))
    dinv = dinv.astype(np.float32)

    # per-core degree-balanced position assignment: node (core c, local r)
    # -> position posmap[c][r] in [0, TPC); similar-degree targets share a
    # group so per-slot edge counts stay even.
    posmap = np.zeros((NCORES, TPC), np.int64)
    for c in range(NCORES):
        degc = deg[c * TPC : (c + 1) * TPC]
        order = np.argsort(-degc, kind="stable")
        posmap[c][order] = np.arange(TPC)

    c_t = tgt // TPC
    pos_t = posmap[c_t, tgt % TPC]
    g_t = pos_t >> 7
    s_t = pos_t & 127

    c_s = src // TPC
    gsrc = c_s * PPC + posmap[c_s, src % TPC]
    chv = gsrc // CHUNK
    lidx = (gsrc % CHUNK).astype(np.int16)

    key = ((c_t * NG + g_t) * NCHUNK + chv) * P + s_t
    cnt = np.bincount(key, minlength=NCORES * NG * NCHUNK * P).reshape(
        NCORES, NG, NCHUNK, P
    )

    # adaptive slabs: grow each slab while ng*nbu stays under SLABB
    gmax = np.maximum(cnt.max(axis=(0, 2, 3)), 1)  # [NG]
    slabs = []
    g0 = 0
    while g0 < NG:
        ng = 1
        mx = int(gmax[g0])
        while g0 + ng < NG:
            m2 = max(mx, int(gmax[g0 + ng]))
            if (ng + 1) * m2 > SLABB:
                break
            mx = m2
            ng += 1
        slabs.append((g0, ng))
        g0 += ng
    nbu = np.zeros(len(slabs), np.int64)
    slaboff = np.zeros(len(slabs), np.int64)
    si_of_g = np.zeros(NG, np.int64)
    gl_of_g = np.zeros(NG, np.int64)
    off = 0
    for si, (gs0, ng) in enumerate(slabs):
        nbu[si] = max(1, int(cnt[:, gs0 : gs0 + ng].max()))
        slaboff[si] = off
        off += NCHUNK * ng * nbu[si] * P
        si_of_g[gs0 : gs0 + ng] = si
        gl_of_g[gs0 : gs0 + ng] = np.arange(ng)
    TOTP = off

    # rank of each edge within its (core, g, ch, s) bucket
    order_e = np.argsort(key, kind="stable")
    ks = key[order_e]
    starts = np.zeros(NCORES * NG * NCHUNK * P, np.int64)
    flat_cnt = cnt.reshape(-1)
    starts[1:] = np.cumsum(flat_cnt)[:-1]
    rank = np.arange(ks.shape[0]) - starts[ks]

    si_e = si_of_g[g_t[order_e]]
    ng_e = np.array([s[1] for s in slabs], np.int64)[si_e]
    pos = (
        slaboff[si_e]
        + ((chv[order_e] * ng_e + gl_of_g[g_t[order_e]]) * nbu[si_e] + rank) * P
        + s_t[order_e]
    )
    core_e = c_t[order_e]
    lidx_e = lidx[order_e]

    idx_list = []
    for c in range(NCORES):
        arr = np.zeros(TOTP, np.int16)
        # default pad: PADIDX (zero row of the chunk)
        arr[:] = PADIDX
        sel = core_e == c
        arr[pos[sel]] = lidx_e[sel]
        idx_list.append(np.tile(arr.reshape(-1, 16).T, (8, 1)).copy())

    sched = {
        "slabs": slabs,
        "nbu": nbu,
        "slaboff": slaboff,
        "TOTP": TOTP,
        "posmap": posmap,
        "dinv": dinv,
    }
    return sched, idx_list, None, None, None


def build_kernel(sched, stage=9, reps=1, shared_ag=True, drop=frozenset()):
    slabs, nbu, slaboff, TOTP = (
        sched["slabs"],
        sched["nbu"],
        sched["slaboff"],
        sched["TOTP"],
    )
    drop = frozenset(drop)

    nc = bacc.Bacc("TRN2", target_bir_lowering=False, num_devices=NCORES)
    xpT = nc.dram_tensor("xpT", [P, PPC], BF16, kind="ExternalInput")
    W1i = nc.dram_tensor("W1i", [IN, HID], BF16, kind="ExternalInput")
    W2i = nc.dram_tensor("W2i", [HID, OUT], BF16, kind="ExternalInput")
    b1i = nc.dram_tensor("b1i", [P, HID], F32, kind="ExternalInput")
    b2i = nc.dram_tensor("b2i", [OUT, 1], F32, kind="ExternalInput")
    dsgi = nc.dram_tensor("dsgi", [P, NG], F32, kind="ExternalInput")
    drwi = nc.dram_tensor("drwi", [P, PPC], BF16, kind="ExternalInput")
    idx = nc.dram_tensor("idx", [P, TOTP // 16], I16, kind="ExternalInput")
    y = nc.dram_tensor("y", [OUT, PPC], F32, kind="ExternalOutput")

    with tile.TileContext(nc) as tc:
        with (
            tc.tile_pool(name="const", bufs=1) as cpool,
            tc.tile_pool(name="sb", bufs=2) as sb,
            tc.tile_pool(name="dram", bufs=1, space="DRAM") as dpool,
            tc.tile_pool(name="psX", bufs=2, space="PSUM") as psX,
        ):
            # ---- constants (loaded once) ----
            W1b = cpool.tile([IN, HID], BF16)
            nc.sync.dma_start(out=W1b[:], in_=W1i[:])
            W2b = cpool.tile([HID, OUT], BF16)
            nc.sync.dma_start(out=W2b[:], in_=W2i[:])
            b1t = cpool.tile([P, HID], F32)
            nc.sync.dma_start(out=b1t[:], in_=b1i[:])
            b2c = cpool.tile([OUT, 1], F32)
            nc.sync.dma_start(out=b2c[:], in_=b2i[:])
            dsg = cpool.tile([P, NG], F32)
            nc.sync.dma_start(out=dsg[:], in_=dsgi[:])
            dsgb = cpool.tile([P, NG], BF16)
            nc.vector.tensor_copy(out=dsgb[:], in_=dsg[:])
            drw = cpool.tile([P, PPC], BF16)
            nc.sync.dma_start(out=drw[:], in_=drwi[:])
            xsb = cpool.tile([P, PPC], BF16)
            nc.sync.dma_start(out=xsb[:], in_=xpT[:])

            use_shared = shared_ag and "ag" not in drop
            addr_space = "Shared" if use_shared else "Local"
            t1in = dpool.tile([PPC, HID], BF16)
            t2in = dpool.tile([PPC, HID], BF16)
            tables1 = [
                dpool.tile([NPAD, HID], BF16, addr_space=addr_space, name=f"tb1_{r}")
                for r in range(reps if use_shared else 1)
            ]
            tables2 = [
                dpool.tile([NPAD, HID], BF16, addr_space=addr_space, name=f"tb2_{r}")
                for r in range(reps if use_shared else 1)
            ]

            if drop:
                ini = sb.tile([P, NG, P], BF16, tag="ini", bufs=1, name="ini")
                nc.vector.memset(ini[:], 0.0)
                for t in (t1in, t2in):
                    nc.sync.dma_start(
                        out=t.rearrange("(n p) f -> p n f", p=P)[:, :, :], in_=ini[:]
                    )
                if "ag" in drop:
                    for t in (tables1[0], tables2[0]):
                        for j in range(NCORES):
                            nc.sync.dma_start(
                                out=t.rearrange(
                                    "(c n p) f -> c p n f", c=NCORES, p=P
                                )[j],
                                in_=ini[:],
                            )

            def agg(table, acc, transpose):
                for si, (gs0, ng) in enumerate(slabs):
                    nb = int(nbu[si])
                    K = ng * nb * P
                    idxs = sb.tile(
                        [P, NCHUNK * K // 16], I16, tag="idxs", bufs=1, name="idxs"
                    )
                    nc.sync.dma_start(
                        out=idxs[:],
                        in_=idx[:, slaboff[si] // 16 : (slaboff[si] + NCHUNK * K) // 16],
                    )
                    if transpose:
                        msgs = sb.tile(
                            [P, NCHUNK * K], BF16, tag="msgs", bufs=1, name="msgsT"
                        )
                    else:
                        msgs = sb.tile(
                            [P, NCHUNK * ng * nb, HID], BF16, tag="msgs", bufs=1,
                            name="msgs",
                        )
                    if "gather" not in drop:
                        for ch in range(NCHUNK):
                            if transpose:
                                o = msgs[:, None, ch * K : (ch + 1) * K]
                            else:
                                o = msgs[:, ch * ng * nb : (ch + 1) * ng * nb, :]
                            nc.gpsimd.dma_gather(
                                o,
                                table[ch * CHUNK : (ch + 1) * CHUNK, :],
                                idxs[:, ch * K // 16 : (ch + 1) * K // 16],
                                K,
                                K,
                                HID,
                                transpose=transpose,
                                single_packet=False,
                            )
                    elif si == 0:
                        nc.vector.memset(msgs[:], 0.0)
                    if "reduce" in drop:
                        continue
                    if transpose:
                        v = msgs.rearrange(
                            "p (c g b s) -> p g s c b", c=NCHUNK, g=ng, b=nb, s=P
                        )
                    else:
                        v = msgs.rearrange(
                            "p (c g b) f -> p g f c b", c=NCHUNK, g=ng, b=nb
                        )
                    nc.vector.tensor_reduce(
                        out=acc[:, gs0 : gs0 + ng, :],
                        in_=v,
                        axis=mybir.AxisListType.XY,
                        op=ALU.add,
                    )

            def body(r):
                table1 = tables1[r if use_shared else 0]
                table2 = tables2[r if use_shared else 0]

                # ---- phase A: table1 = xpre @ W1 ----
                # feature-major matmul (lhsT=W1, rhs=x^T) then a transposing
                # strided DMA writes node-major rows to DRAM.
                if stage >= 1:
                    NCHK = 512
                    for j0 in range(0, PPC, NCHK):
                        n = min(NCHK, PPC - j0)
                        ps = psX.tile([P, NCHK], F32, tag="px", name="ps")
                        nc.tensor.matmul(
                            out=ps[:, :n], lhsT=W1b[:], rhs=xsb[:, j0 : j0 + n],
                            start=True, stop=True,
                        )
                        t1f = sb.tile([P, NCHK], BF16, tag="t1f", name="t1f")
                        nc.scalar.activation(t1f[:, :n], ps[:, :n], AF.Copy)
                        nc.sync.dma_start(
                            out=t1in[j0 : j0 + n, :].rearrange("n f -> f n"),
                            in_=t1f[:, :n],
                        )
                if stage >= 2 and "ag" not in drop:
                    nc.gpsimd.collective_compute(
                        "AllGather", ALU.bypass, ins=[t1in[:]], outs=[table1[:]],
                        replica_groups=[list(range(NCORES))],
                    )

                # ---- L1 aggregation (slot-major acc1) + epilogue ----
                if stage >= 3:
                    acc1 = sb.tile([P, NG, HID], F32, tag="acc", bufs=1, name="acc1")
                    agg(table1, acc1, transpose=False)
                    if "epi" not in drop:
                        nc.vector.tensor_tensor(
                            out=acc1[:], in0=acc1[:],
                            in1=dsg[:, :, None].to_broadcast([P, NG, HID]),
                            op=ALU.mult,
                        )
                        nc.vector.tensor_tensor(
                            out=acc1[:], in0=acc1[:],
                            in1=b1t[:, None, :].to_broadcast([P, NG, HID]),
                            op=ALU.add,
                        )
                        y1t = sb.tile([P, NG, HID], BF16, tag="bfb", bufs=1, name="y1t")
                        nc.scalar.activation(y1t[:], acc1[:], AF.Relu)
                        nc.vector.tensor_tensor(
                            out=y1t[:], in0=y1t[:],
                            in1=dsgb[:, :, None].to_broadcast([P, NG, HID]),
                            op=ALU.mult,
                        )
                        nc.sync.dma_start(
                            out=t2in.rearrange("(n p) f -> p n f", p=P)[:, :, :],
                            in_=y1t[:],
                        )
                if stage >= 5 and "ag" not in drop:
                    nc.gpsimd.collective_compute(
                        "AllGather", ALU.bypass, ins=[t2in[:]], outs=[table2[:]],
                        replica_groups=[list(range(NCORES))],
                    )

                # ---- L2 aggregation (feature-major acc2) + projection ----
                if stage >= 6:
                    acc2 = sb.tile([P, NG, P], F32, tag="acc", bufs=1, name="acc2")
                    agg(table2, acc2, transpose=True)
                    if "epi" in drop:
                        return
                    a2s = sb.tile([P, PPC], BF16, tag="bfb", bufs=1, name="a2s")
                    nc.vector.tensor_tensor(
                        out=a2s[:],
                        in0=acc2.rearrange("p g s -> p (g s)"),
                        in1=drw[:],
                        op=ALU.mult,
                    )
                    NCHK = 512
                    YB = 4 * NCHK
                    for j0 in range(0, PPC, NCHK):
                        n = min(NCHK, PPC - j0)
                        if j0 % YB == 0:
                            yb0 = j0
                            ych = sb.tile([OUT, YB], F32, tag="ych", name="ych")
                        psY = psX.tile([OUT, NCHK], F32, tag="py", name="psY")
                        nc.tensor.matmul(
                            out=psY[:, :n], lhsT=W2b[:], rhs=a2s[:, j0 : j0 + n],
                            start=True, stop=True,
                        )
                        nc.vector.tensor_tensor(
                            out=ych[:, j0 - yb0 : j0 - yb0 + n], in0=psY[:, :n],
                            in1=b2c[:].to_broadcast([OUT, n]), op=ALU.add,
                        )
                        if j0 + n == PPC or (j0 + NCHK) % YB == 0:
                            nc.sync.dma_start(
                                out=y[:, yb0 : j0 + n], in_=ych[:, : j0 + n - yb0]
                            )

            for r in range(reps):
                body(r)

    nc.finalize()
    return nc


def make_in_maps(inputs, sched, idx_list, _colv=None, _dego=None):
    x = np.asarray(inputs["x"], np.float32)
    W1 = np.asarray(inputs["W1"], np.float32)
    W2 = np.asarray(inputs["W2"], np.float32)
    b1 = np.asarray(inputs["b1"], np.float32)
    b2 = np.asarray(inputs["b2"], np.float32)
    dinv = sched["dinv"]
    posmap = sched["posmap"]

    xpre = x * dinv[:, None]
    b1r = np.tile(b1[None, :], (P, 1)).astype(np.float32)
    in_maps = []
    for c in range(NCORES):
        # position-ordered per-core node data
        inv = np.argsort(posmap[c])  # position -> local node
        xs = np.zeros((P, PPC), ml_dtypes.bfloat16)
        xs[:, :TPC] = xpre[c * TPC : (c + 1) * TPC][inv].T.astype(ml_dtypes.bfloat16)
        dpos = np.zeros(PPC, np.float32)
        dpos[:TPC] = dinv[c * TPC : (c + 1) * TPC][inv]
        dsg = np.ascontiguousarray(dpos.reshape(NG, P).T)  # [slot, group]
        drw = np.tile(dpos.astype(ml_dtypes.bfloat16)[None, :], (P, 1))
        in_maps.append(
            {
                "xpT": xs,
                "W1i": W1.astype(ml_dtypes.bfloat16),
                "W2i": W2.astype(ml_dtypes.bfloat16),
                "b1i": b1r,
                "b2i": b2[:, None].astype(np.float32),
                "dsgi": dsg,
                "drwi": drw,
                "idx": idx_list[c],
            }
        )
    return in_maps


def assemble_output(results, sched):
    posmap = sched["posmap"]
    outs = []
    for c in range(NCORES):
        yc = results[c]["y"]  # [OUT, PPC] position-ordered
        outs.append(yc[:, posmap[c]].T)  # [TPC, OUT] node-ordered
    return np.concatenate(outs, axis=0)


def kernel(**inputs):
    sched, idx_list, *_ = host_prep(inputs["edge_index"])
    nc = build_kernel(sched)
    in_maps = make_in_maps(inputs, sched, idx_list)
    res = run_bass_kernel_spmd(nc, in_maps, core_ids=list(range(NCORES)))
    return assemble_output(res.results, sched)
